# revision 9
# baseline (speedup 1.0000x reference)
"""Trainium2 Bass kernel: 16-head RoPE attention (B=2, L=2048, HIDDEN=1024).

Sharding: 8 cores = 2 batches x 4 head-groups (4 heads per core).
Each core computes q/k/v projections for its 4 heads (feature-major),
RoPE, scores-transposed [k,q] per head, exp (no max subtraction --
scores are ~N(0,1)), PV with a ones-column in V to get softmax sums,
normalization, and a partial output projection [1024, 2048].
Host sums the 4 partials per batch and transposes back.
"""

import numpy as np
from contextlib import ExitStack

from concourse import bacc, tile, mybir
from concourse.bass import ts
from concourse.bass_utils import run_bass_kernel_spmd

HIDDEN = 1024
HEADS = 16
HD = 64
L = 2048
B = 2
BASE = 10000.0

P = 128
E_LOCAL = 256          # 4 heads per core
N_PAIRS = 2            # head pairs per core (2 heads on 128 partitions)
HC = HIDDEN // P       # 8 hidden chunks
TC = 512               # token chunk (matmul free dim)
N_TC = L // TC         # 4
N_TT = L // P          # 16 token tiles (for v / k-tiles)
SCALE = 1.0 / 8.0      # 1/sqrt(HD)

F32 = mybir.dt.float32
F32R = mybir.dt.float32r
AF = mybir.ActivationFunctionType
ALU = mybir.AluOpType


def r(ap):
    """View an fp32 AP as float32r for full-rate PE matmuls."""
    return ap.bitcast(F32R)


def build_program(debug=False):
    nc = bacc.Bacc(None, target_bir_lowering=False)
    names = {}
    with tile.TileContext(nc) as tc:
        ctx = ExitStack()
        with ctx:
            dram = ctx.enter_context(tc.tile_pool(name="dram", bufs=1, space="DRAM"))
            xT_d = dram.tile([HIDDEN, L], F32, kind="ExternalInput", name="xT")
            wq_d = dram.tile([HIDDEN, E_LOCAL], F32, kind="ExternalInput", name="wq")
            wk_d = dram.tile([HIDDEN, E_LOCAL], F32, kind="ExternalInput", name="wk")
            wv_d = dram.tile([HIDDEN, E_LOCAL], F32, kind="ExternalInput", name="wv")
            wo_d = dram.tile([E_LOCAL, HIDDEN], F32, kind="ExternalInput", name="wo")
            cos_d = dram.tile([P, L], F32, kind="ExternalInput", name="cosT")
            sin_d = dram.tile([P, L], F32, kind="ExternalInput", name="sinT")
            out_d = dram.tile([HIDDEN, L], F32, kind="ExternalOutput", name="outT")
            if debug:
                dbg_q = dram.tile([P, L], F32, kind="ExternalOutput", name="dbg_q")
                dbg_k = dram.tile([P, L], F32, kind="ExternalOutput", name="dbg_k")
                dbg_v = dram.tile([P, N_TT * 4 * (HD + 1)], F32, kind="ExternalOutput", name="dbg_v")
                dbg_o = dram.tile([P, L], F32, kind="ExternalOutput", name="dbg_o")
                dbg_ot = dram.tile([HD + 1, 2 * TC], F32, kind="ExternalOutput", name="dbg_ot")
                dbg_inv = dram.tile([1, 2 * TC], F32, kind="ExternalOutput", name="dbg_inv")
                dbg_bsum = dram.tile([HD, 2 * TC], F32, kind="ExternalOutput", name="dbg_bsum")
                names["dbg"] = [t.tensor.name for t in (dbg_q, dbg_k, dbg_v, dbg_o, dbg_ot, dbg_inv, dbg_bsum)]
            names["in"] = ["xT", "wq", "wk", "wv", "wo", "cosT", "sinT"]
            names["out"] = "outT"
            names["in"] = [t.tensor.name for t in (xT_d, wq_d, wk_d, wv_d, wo_d, cos_d, sin_d)]
            names["out"] = out_d.tensor.name

            # ---------------- persistent SBUF ----------------
            const = ctx.enter_context(tc.tile_pool(name="const", bufs=1))
            wq_sb = const.tile([P, HC, E_LOCAL], F32R)
            wk_sb = const.tile([P, HC, E_LOCAL], F32R)
            wv_sb = const.tile([P, HC, E_LOCAL], F32R)
            wo_sb = const.tile([P, 2, HIDDEN], F32R)
            cos_sb = const.tile([P, L], F32)
            sin_sb = const.tile([P, L], F32)
            nc.sync.dma_start(wq_sb[:], r(wq_d[:].rearrange("(c p) e -> p c e", p=P)))
            nc.sync.dma_start(wk_sb[:], r(wk_d[:].rearrange("(c p) e -> p c e", p=P)))
            nc.sync.dma_start(wv_sb[:], r(wv_d[:].rearrange("(c p) e -> p c e", p=P)))
            nc.sync.dma_start(wo_sb[:], r(wo_d[:].rearrange("(c p) f -> p c f", p=P)))
            nc.sync.dma_start(cos_sb[:], cos_d[:])
            nc.sync.dma_start(sin_sb[:], sin_d[:])

            # rope'd q and k, feature-major: per pair [128, L]
            qkro = ctx.enter_context(tc.tile_pool(name="qkro", bufs=1))
            q_ro = [qkro.tile([P, L], F32R, name=f"q_ro{p}") for p in range(N_PAIRS)]
            k_ro = [qkro.tile([P, L], F32R, name=f"k_ro{p}") for p in range(N_PAIRS)]
            # v token-major with ones columns: [128 tok, tt, 4*65]
            v_all = qkro.tile([P, N_TT, 4 * (HD + 1)], F32R)
            v4 = v_all[:].rearrange("p t (g c) -> p t g c", g=4)
            ones_sb = qkro.tile([P, N_TT], F32)
            nc.vector.memset(ones_sb[:], 1.0)
            for g in range(4):
                nc.vector.tensor_copy(
                    v_all[:, :, g * (HD + 1) + HD : g * (HD + 1) + HD + 1],
                    ones_sb[:].rearrange("p (a b) -> p a b", b=1),
                )
            # normalized attention output, feature-major per pair [128, L]
            o_sb = [qkro.tile([P, L], F32R, name=f"o_sb{p}") for p in range(N_PAIRS)]

            # ---------------- projections ----------------
            xpool = ctx.enter_context(tc.tile_pool(name="xpool", bufs=10))
            rope_t = ctx.enter_context(tc.tile_pool(name="rope", bufs=2))

            def rope_chunk(dst, ps_tile, t, is_q):
                """psum [128, TC] -> dst[:, t*TC:(t+1)*TC] with RoPE applied."""
                raw = rope_t.tile([P, TC], F32, name="raw")
                shuf = rope_t.tile([P, TC], F32, name="shuf")
                t1 = rope_t.tile([P, TC], F32, name="t1")
                t2 = rope_t.tile([P, TC], F32, name="t2")
                nc.vector.tensor_copy(raw[:], ps_tile[:])
                # swap 32-partition halves within each 64-row head block
                for a, b in ((0, 32), (32, 0), (64, 96), (96, 64)):
                    nc.sync.dma_start(shuf[a : a + 32, :], raw[b : b + 32, :])
                cs = cos_sb[:, ts(t, TC)]
                sn = sin_sb[:, ts(t, TC)]
                nc.vector.tensor_mul(t1[:], raw[:], cs)
                nc.gpsimd.tensor_mul(t2[:], shuf[:], sn)
                nc.vector.tensor_add(dst[:, ts(t, TC)], t1[:], t2[:])

            with tc.tile_pool(name="ps_qk", bufs=2, space="PSUM") as ps_qk:
                with tc.tile_pool(name="ps_v", bufs=2, space="PSUM") as ps_v:
                    # pair-0 q/k projections first so attention can start early;
                    # v (both pairs) next; pair-1 q/k last.
                    for pair in range(N_PAIRS):
                        for t in range(N_TC):
                            xts = []
                            for h in range(HC):
                                xt = xpool.tile([P, TC], F32R, name="xt")
                                nc.sync.dma_start(
                                    xt[:], r(xT_d[ts(h, P), ts(t, TC)])
                                )
                                xts.append(xt)
                            qp = ps_qk.tile([P, TC], F32, name="qp")
                            kp = ps_qk.tile([P, TC], F32, name="kp")
                            for h in range(HC):
                                nc.tensor.matmul(
                                    qp[:],
                                    wq_sb[:, h, ts(pair, P)],
                                    xts[h][:],
                                    start=(h == 0),
                                    stop=(h == HC - 1),
                                )
                            for h in range(HC):
                                nc.tensor.matmul(
                                    kp[:],
                                    wk_sb[:, h, ts(pair, P)],
                                    xts[h][:],
                                    start=(h == 0),
                                    stop=(h == HC - 1),
                                )
                            rope_chunk(q_ro[pair], qp, t, True)
                            rope_chunk(k_ro[pair], kp, t, False)

                        if pair == 0:
                            # v projection (token-major, both pairs at once: N=256)
                            for t in range(N_TC):
                                xts = []
                                for h in range(HC):
                                    xt = xpool.tile([P, TC], F32R, name="xt")
                                    nc.sync.dma_start(
                                        xt[:], r(xT_d[ts(h, P), ts(t, TC)])
                                    )
                                    xts.append(xt)
                                for s in range(TC // P):  # 4 token tiles per chunk
                                    tt = t * (TC // P) + s
                                    vp = ps_v.tile([P, E_LOCAL], F32, name="vp")
                                    for h in range(HC):
                                        nc.tensor.matmul(
                                            vp[:],
                                            xts[h][:, ts(s, P)],
                                            wv_sb[:, h, :],
                                            start=(h == 0),
                                            stop=(h == HC - 1),
                                        )
                                    # scatter into v_all with ones-gaps
                                    for pr in range(N_PAIRS):
                                        src = vp[:, ts(pr, P)].rearrange(
                                            "p (g c) -> p g c", g=2
                                        )
                                        dst = v4[:, tt, 2 * pr : 2 * pr + 2, 0:HD]
                                        nc.vector.tensor_copy(dst, src)

            # ---------------- attention ----------------
            expp = ctx.enter_context(tc.tile_pool(name="expp", bufs=2 if debug else 3))
            nrm = ctx.enter_context(tc.tile_pool(name="nrm", bufs=2))
            with tc.tile_pool(name="ps_ot", bufs=2, space="PSUM") as ps_ot:
                with tc.tile_pool(name="ps_st", bufs=2, space="PSUM") as ps_st:
                    for pair in range(N_PAIRS):
                        for c in range(N_TC):
                            ot = ps_ot.tile([HD + 1, 2 * TC], F32, name="ot")
                            for kt in range(N_TT):
                                st = ps_st.tile([P, 2 * TC], F32, name="st")
                                nc.tensor.matmul(
                                    st[:, 0:TC],
                                    k_ro[pair][0:HD, ts(kt, P)],
                                    q_ro[pair][0:HD, ts(c, TC)],
                                    start=True,
                                    stop=True,
                                )
                                nc.tensor.matmul(
                                    st[:, TC : 2 * TC],
                                    k_ro[pair][HD:P, ts(kt, P)],
                                    q_ro[pair][HD:P, ts(c, TC)],
                                    start=True,
                                    stop=True,
                                    tile_position=(64, 0),
                                )
                                ex = expp.tile([P, 2 * TC], F32R, name="ex")
                                nc.scalar.activation(ex[:], st[:], AF.Exp, scale=SCALE)
                                for hd_i in range(2):
                                    g = 2 * pair + hd_i
                                    nc.tensor.matmul(
                                        ot[:, ts(hd_i, TC)],
                                        v_all[:, kt, g * (HD + 1) : (g + 1) * (HD + 1)],
                                        ex[:, ts(hd_i, TC)],
                                        start=(kt == 0),
                                        stop=(kt == N_TT - 1),
                                    )
                            # normalize: rows 0..63 are o, row 64 is sum(exp)
                            srow = nrm.tile([HD + 1, 2 * TC], F32, name="srow")
                            nc.vector.tensor_copy(srow[HD : HD + 1, :], ot[HD : HD + 1, :])
                            s32 = nrm.tile([32, 2 * TC // 32], F32, name="s32")
                            nc.sync.dma_start(
                                s32[:],
                                srow[HD : HD + 1, :].rearrange("p (a b) -> p a b", a=32),
                            )
                            nc.vector.reciprocal(s32[:], s32[:])
                            invrow = nrm.tile([1, 2 * TC], F32, name="invrow")
                            nc.sync.dma_start(
                                invrow[:].rearrange("p (a b) -> p a b", a=32), s32[:]
                            )
                            bsum = nrm.tile([HD, 2 * TC], F32, name="bsum")
                            nc.gpsimd.partition_broadcast(bsum[:], invrow[:])
                            if debug and pair == 0 and c == 0:
                                ot_cp = nrm.tile([HD + 1, 2 * TC], F32, name="ot_cp", bufs=1)
                                nc.vector.tensor_copy(ot_cp[:], ot[:])
                                nc.sync.dma_start(dbg_ot[:], ot_cp[:])
                                nc.sync.dma_start(dbg_inv[:], invrow[:])
                                nc.sync.dma_start(dbg_bsum[:], bsum[:])
                            for hd_i in range(2):
                                dsts = o_sb[pair]
                                if hd_i == 0:
                                    nc.vector.tensor_mul(
                                        dsts[0:HD, ts(c, TC)],
                                        ot[0:HD, ts(hd_i, TC)],
                                        bsum[:, ts(hd_i, TC)],
                                    )
                                else:
                                    onrm = nrm.tile([HD, TC], F32, name="onrm")
                                    nc.vector.tensor_mul(
                                        onrm[:],
                                        ot[0:HD, ts(hd_i, TC)],
                                        bsum[:, ts(hd_i, TC)],
                                    )
                                    nc.sync.dma_start(
                                        dsts[HD:P, ts(c, TC)], r(onrm[:])
                                    )

            if debug:
                nc.sync.dma_start(dbg_q[:], q_ro[0][:].bitcast(F32))
                nc.sync.dma_start(dbg_k[:], k_ro[0][:].bitcast(F32))
                nc.sync.dma_start(dbg_v[:], v_all[:].rearrange("p a b -> p (a b)").bitcast(F32))
                nc.sync.dma_start(dbg_o[:], o_sb[0][:].bitcast(F32))

            # ---------------- output projection ----------------
            outst = ctx.enter_context(tc.tile_pool(name="outst", bufs=2))
            with tc.tile_pool(name="ps_o", bufs=2, space="PSUM") as ps_o:
                for fc in range(HC):
                    for t in range(N_TC):
                        op = ps_o.tile([P, TC], F32, name="op")
                        for pair in range(N_PAIRS):
                            nc.tensor.matmul(
                                op[:],
                                wo_sb[:, pair, ts(fc, P)],
                                o_sb[pair][:, ts(t, TC)],
                                start=(pair == 0),
                                stop=(pair == N_PAIRS - 1),
                            )
                        ob = outst.tile([P, TC], F32, name="ob")
                        nc.vector.tensor_copy(ob[:], op[:])
                        nc.sync.dma_start(out_d[ts(fc, P), ts(t, TC)], ob[:])

    nc.compile()
    return nc, names


_CACHE = {}


def _get_program():
    if "prog" not in _CACHE:
        _CACHE["prog"] = build_program()
    return _CACHE["prog"]


def _rope_tables():
    inv_freq = 1.0 / (BASE ** (np.arange(0, HD, 2, dtype=np.float64) / HD))
    t = np.arange(L, dtype=np.float64)
    freqs = np.outer(t, inv_freq)            # [L, 32]
    emb = np.concatenate((freqs, freqs), -1)  # [L, 64]
    cos = np.cos(emb).T.astype(np.float32)    # [64, L]
    sin = np.sin(emb).T.astype(np.float32)    # [64, L]
    sin_signed = sin.copy()
    sin_signed[: HD // 2] *= -1.0             # rotate_half sign baked in
    cosT = np.ascontiguousarray(np.concatenate([cos, cos], 0))      # [128, L]
    sinT = np.ascontiguousarray(np.concatenate([sin_signed, sin_signed], 0))
    return cosT, sinT


def kernel(x, Wq, Wk, Wv, Wo):
    x = np.asarray(x, dtype=np.float32)
    Wq = np.asarray(Wq, dtype=np.float32)
    Wk = np.asarray(Wk, dtype=np.float32)
    Wv = np.asarray(Wv, dtype=np.float32)
    Wo = np.asarray(Wo, dtype=np.float32)

    nc, names = _get_program()
    cosT, sinT = _rope_tables()

    in_maps = []
    for core in range(8):
        b = core // 4
        g = core % 4
        es = slice(g * E_LOCAL, (g + 1) * E_LOCAL)
        xT = np.ascontiguousarray(x[b].T)                  # [1024, L]
        m = {
            names["in"][0]: xT,
            names["in"][1]: np.ascontiguousarray(Wq[es, :].T),   # [1024, 256]
            names["in"][2]: np.ascontiguousarray(Wk[es, :].T),
            names["in"][3]: np.ascontiguousarray(Wv[es, :].T),
            names["in"][4]: np.ascontiguousarray(Wo[:, es].T),   # [256, 1024]
            names["in"][5]: cosT,
            names["in"][6]: sinT,
        }
        in_maps.append(m)

    res = run_bass_kernel_spmd(nc, in_maps, core_ids=list(range(8)))

    out = np.zeros((B, L, HIDDEN), dtype=np.float32)
    for b in range(B):
        acc = np.zeros((HIDDEN, L), dtype=np.float32)
        for g in range(4):
            acc += res.results[b * 4 + g][names["out"]]
        out[b] = acc.T
    return out


# revision 24
# speedup vs baseline: 21240.7074x; 21240.7074x over previous
"""Trainium2 Bass kernel: 16-head RoPE attention (B=2, L=2048, HIDDEN=1024).

Sharding: 8 cores = 2 batches x 4 head-groups (4 heads per core).
Each core computes q/k/v projections for its 4 heads (feature-major),
RoPE, scores-transposed [k,q] per head, exp (no max subtraction --
scores are ~N(0,1)), PV with a ones-column in V to get softmax sums,
normalization, and a partial output projection [1024, 2048].
Host sums the 4 partials per batch and transposes back.
"""

import numpy as np
from contextlib import ExitStack

from concourse import bacc, tile, mybir
from concourse.bass import ts
from concourse.bass_utils import run_bass_kernel_spmd

HIDDEN = 1024
HEADS = 16
HD = 64
L = 2048
B = 2
BASE = 10000.0

P = 128
E_LOCAL = 256          # 4 heads per core
N_PAIRS = 2            # head pairs per core (2 heads on 128 partitions)
HC = HIDDEN // P       # 8 hidden chunks
TC = 512               # token chunk (matmul free dim)
N_TC = L // TC         # 4
N_TT = L // P          # 16 token tiles (for v / k-tiles)
SCALE = 1.0 / 8.0      # 1/sqrt(HD)

F32 = mybir.dt.float32
F32R = mybir.dt.float32r
AF = mybir.ActivationFunctionType
ALU = mybir.AluOpType


def r(ap):
    """View an fp32 AP as float32r for full-rate PE matmuls."""
    return ap.bitcast(F32R)


def build_program(debug=False):
    nc = bacc.Bacc(None, target_bir_lowering=False)
    names = {}
    with tile.TileContext(nc) as tc:
        ctx = ExitStack()
        with ctx:
            dram = ctx.enter_context(tc.tile_pool(name="dram", bufs=1, space="DRAM"))
            xT_d = dram.tile([HIDDEN, L], F32, kind="ExternalInput", name="xT")
            wq_d = dram.tile([HIDDEN, E_LOCAL], F32, kind="ExternalInput", name="wq")
            wk_d = dram.tile([HIDDEN, E_LOCAL], F32, kind="ExternalInput", name="wk")
            wv_d = dram.tile([HIDDEN, E_LOCAL], F32, kind="ExternalInput", name="wv")
            wo_d = dram.tile([E_LOCAL, HIDDEN], F32, kind="ExternalInput", name="wo")
            cos_d = dram.tile([P, L], F32, kind="ExternalInput", name="cosT")
            sin_d = dram.tile([P, L], F32, kind="ExternalInput", name="sinT")
            out_d = dram.tile([HIDDEN, L], F32, kind="ExternalOutput", name="outT")
            if debug:
                dbg_q = dram.tile([P, L], F32, kind="ExternalOutput", name="dbg_q")
                dbg_k = dram.tile([P, L], F32, kind="ExternalOutput", name="dbg_k")
                dbg_v = dram.tile([P, N_TT * 4 * (HD + 1)], F32, kind="ExternalOutput", name="dbg_v")
                dbg_o = dram.tile([P, L], F32, kind="ExternalOutput", name="dbg_o")
                dbg_ot = dram.tile([HD + 1, 2 * TC], F32, kind="ExternalOutput", name="dbg_ot")
                dbg_inv = dram.tile([1, 2 * TC], F32, kind="ExternalOutput", name="dbg_inv")
                dbg_bsum = dram.tile([HD, 2 * TC], F32, kind="ExternalOutput", name="dbg_bsum")
                names["dbg"] = [t.tensor.name for t in (dbg_q, dbg_k, dbg_v, dbg_o, dbg_ot, dbg_inv, dbg_bsum)]
            names["in"] = ["xT", "wq", "wk", "wv", "wo", "cosT", "sinT"]
            names["out"] = "outT"
            names["in"] = [t.tensor.name for t in (xT_d, wq_d, wk_d, wv_d, wo_d, cos_d, sin_d)]
            names["out"] = out_d.tensor.name

            # ---------------- persistent SBUF ----------------
            const = ctx.enter_context(tc.tile_pool(name="const", bufs=1))
            wq_sb = const.tile([P, HC, E_LOCAL], F32R)
            wk_sb = const.tile([P, HC, E_LOCAL], F32R)
            wv_sb = const.tile([P, HC, E_LOCAL], F32R)
            wo_sb = const.tile([P, 2, HIDDEN], F32R)
            cos_sb = const.tile([P, L], F32)
            sin_sb = const.tile([P, L], F32)
            nc.sync.dma_start(wq_sb[:], r(wq_d[:].rearrange("(c p) e -> p c e", p=P)))
            nc.sync.dma_start(wk_sb[:], r(wk_d[:].rearrange("(c p) e -> p c e", p=P)))

            # rope'd q and k, feature-major: per pair [128, L]
            qkro = ctx.enter_context(tc.tile_pool(name="qkro", bufs=1))
            q_ro = [qkro.tile([P, L], F32R, name=f"q_ro{p}") for p in range(N_PAIRS)]
            k_ro = [qkro.tile([P, L], F32R, name=f"k_ro{p}") for p in range(N_PAIRS)]
            # v token-major with ones columns: [128 tok, tt, 4*65]
            v_all = qkro.tile([P, N_TT, 4 * (HD + 1)], F32R)
            v4 = v_all[:].rearrange("p t (g c) -> p t g c", g=4)
            ones_sb = qkro.tile([P, N_TT], F32)
            nc.vector.memset(ones_sb[:], 1.0)
            for g in range(4):
                nc.vector.tensor_copy(
                    v_all[:, :, g * (HD + 1) + HD : g * (HD + 1) + HD + 1],
                    ones_sb[:].rearrange("p (a b) -> p a b", b=1),
                )
            # normalized attention output, feature-major per pair [128, L]
            o_sb = [qkro.tile([P, L], F32R, name=f"o_sb{p}") for p in range(N_PAIRS)]

            # ---------------- projections ----------------
            xpool = ctx.enter_context(tc.tile_pool(name="xpool", bufs=13))
            rope_t = ctx.enter_context(tc.tile_pool(name="rope", bufs=2))

            def rope_chunk(dst, ps_tile, t, is_q):
                """psum [128, TC] -> dst[:, t*TC:(t+1)*TC] with RoPE applied."""
                raw = rope_t.tile([P, TC], F32, name="raw")
                shuf = rope_t.tile([P, TC], F32, name="shuf")
                t1 = rope_t.tile([P, TC], F32, name="t1")
                t2 = rope_t.tile([P, TC], F32, name="t2")
                nc.vector.tensor_copy(raw[:], ps_tile[:])
                # swap 32-partition halves within each 64-row head block
                for a, b in ((0, 32), (32, 0), (64, 96), (96, 64)):
                    nc.gpsimd.dma_start(shuf[a : a + 32, :], raw[b : b + 32, :])
                cs = cos_sb[:, ts(t, TC)]
                sn = sin_sb[:, ts(t, TC)]
                nc.vector.tensor_mul(t1[:], raw[:], cs)
                nc.gpsimd.tensor_mul(t2[:], shuf[:], sn)
                nc.vector.tensor_add(dst[:, ts(t, TC)], t1[:], t2[:])

            with tc.tile_pool(name="ps_qk", bufs=2, space="PSUM") as ps_qk:
                with tc.tile_pool(name="ps_v", bufs=2, space="PSUM") as ps_v:
                    # pair-0 q/k projections first so attention can start early;
                    # v (both pairs) next; pair-1 q/k last.
                    for pair in range(N_PAIRS):
                        for t in range(N_TC):
                            xts = []
                            for h in range(HC):
                                xt = xpool.tile([P, TC], F32R, name="xt")
                                nc.sync.dma_start(
                                    xt[:], r(xT_d[ts(h, P), ts(t, TC)])
                                )
                                xts.append(xt)
                            qp = ps_qk.tile([P, TC], F32, name="qp")
                            kp = ps_qk.tile([P, TC], F32, name="kp")
                            for h in range(HC):
                                nc.tensor.matmul(
                                    qp[:],
                                    wq_sb[:, h, ts(pair, P)],
                                    xts[h][:],
                                    start=(h == 0),
                                    stop=(h == HC - 1),
                                )
                            for h in range(HC):
                                nc.tensor.matmul(
                                    kp[:],
                                    wk_sb[:, h, ts(pair, P)],
                                    xts[h][:],
                                    start=(h == 0),
                                    stop=(h == HC - 1),
                                )
                            rope_chunk(q_ro[pair], qp, t, True)
                            rope_chunk(k_ro[pair], kp, t, False)

                        if pair == 0:
                            # v projection (token-major, both pairs at once: N=256)
                            for t in range(N_TC):
                                xts = []
                                for h in range(HC):
                                    xt = xpool.tile([P, TC], F32R, name="xt")
                                    nc.sync.dma_start(
                                        xt[:], r(xT_d[ts(h, P), ts(t, TC)])
                                    )
                                    xts.append(xt)
                                for s in range(TC // P):  # 4 token tiles per chunk
                                    tt = t * (TC // P) + s
                                    vp = ps_v.tile([P, E_LOCAL], F32, name="vp")
                                    for h in range(HC):
                                        nc.tensor.matmul(
                                            vp[:],
                                            xts[h][:, ts(s, P)],
                                            wv_sb[:, h, :],
                                            start=(h == 0),
                                            stop=(h == HC - 1),
                                        )
                                    # scatter into v_all with ones-gaps
                                    for pr in range(N_PAIRS):
                                        src = vp[:, ts(pr, P)].rearrange(
                                            "p (g c) -> p g c", g=2
                                        )
                                        dst = v4[:, tt, 2 * pr : 2 * pr + 2, 0:HD]
                                        nc.vector.tensor_copy(dst, src)

            # ---------------- attention ----------------
            expp = ctx.enter_context(tc.tile_pool(name="expp", bufs=2 if debug else 3))
            nrm = ctx.enter_context(tc.tile_pool(name="nrm", bufs=2))
            with tc.tile_pool(name="ps_ot", bufs=2, space="PSUM") as ps_ot:
                with tc.tile_pool(name="ps_st", bufs=2, space="PSUM") as ps_st:
                    for pair in range(N_PAIRS):
                        for c in range(N_TC):
                            ot = ps_ot.tile([HD + 1, 2 * TC], F32, name="ot")
                            for kt in range(N_TT):
                                st = ps_st.tile([P, 2 * TC], F32, name="st")
                                nc.tensor.matmul(
                                    st[:, 0:TC],
                                    k_ro[pair][0:HD, ts(kt, P)],
                                    q_ro[pair][0:HD, ts(c, TC)],
                                    start=True,
                                    stop=True,
                                )
                                nc.tensor.matmul(
                                    st[:, TC : 2 * TC],
                                    k_ro[pair][HD:P, ts(kt, P)],
                                    q_ro[pair][HD:P, ts(c, TC)],
                                    start=True,
                                    stop=True,
                                    tile_position=(64, 0),
                                )
                                ex = expp.tile([P, 2 * TC], F32R, name="ex")
                                nc.scalar.activation(ex[:], st[:], AF.Exp, scale=SCALE)
                                for hd_i in range(2):
                                    g = 2 * pair + hd_i
                                    nc.tensor.matmul(
                                        ot[:, ts(hd_i, TC)],
                                        v_all[:, kt, g * (HD + 1) : (g + 1) * (HD + 1)],
                                        ex[:, ts(hd_i, TC)],
                                        start=(kt == 0),
                                        stop=(kt == N_TT - 1),
                                    )
                            # normalize: rows 0..63 are o, row 64 is sum(exp)
                            srow = nrm.tile([HD + 1, 2 * TC], F32, name="srow")
                            nc.vector.tensor_copy(srow[HD : HD + 1, :], ot[HD : HD + 1, :])
                            s32 = nrm.tile([32, 2 * TC // 32], F32, name="s32")
                            nc.sync.dma_start(
                                s32[:],
                                srow[HD : HD + 1, :].rearrange("p (a b) -> p a b", a=32),
                            )
                            nc.vector.reciprocal(s32[:], s32[:])
                            invrow = nrm.tile([1, 2 * TC], F32, name="invrow")
                            nc.sync.dma_start(
                                invrow[:].rearrange("p (a b) -> p a b", a=32), s32[:]
                            )
                            bsum = nrm.tile([HD, 2 * TC], F32, name="bsum")
                            nc.gpsimd.partition_broadcast(bsum[:], invrow[:])
                            if debug and pair == 0 and c == 0:
                                ot_cp = nrm.tile([HD + 1, 2 * TC], F32, name="ot_cp", bufs=1)
                                nc.vector.tensor_copy(ot_cp[:], ot[:])
                                nc.sync.dma_start(dbg_ot[:], ot_cp[:])
                                nc.sync.dma_start(dbg_inv[:], invrow[:])
                                nc.sync.dma_start(dbg_bsum[:], bsum[:])
                            for hd_i in range(2):
                                dsts = o_sb[pair]
                                if hd_i == 0:
                                    nc.vector.tensor_mul(
                                        dsts[0:HD, ts(c, TC)],
                                        ot[0:HD, ts(hd_i, TC)],
                                        bsum[:, ts(hd_i, TC)],
                                    )
                                else:
                                    onrm = nrm.tile([HD, TC], F32, name="onrm")
                                    nc.vector.tensor_mul(
                                        onrm[:],
                                        ot[0:HD, ts(hd_i, TC)],
                                        bsum[:, ts(hd_i, TC)],
                                    )
                                    nc.sync.dma_start(
                                        dsts[HD:P, ts(c, TC)], r(onrm[:])
                                    )

            if debug:
                nc.sync.dma_start(dbg_q[:], q_ro[0][:].bitcast(F32))
                nc.sync.dma_start(dbg_k[:], k_ro[0][:].bitcast(F32))
                nc.sync.dma_start(dbg_v[:], v_all[:].rearrange("p a b -> p (a b)").bitcast(F32))
                nc.sync.dma_start(dbg_o[:], o_sb[0][:].bitcast(F32))

            # ---------------- output projection ----------------
            outst = ctx.enter_context(tc.tile_pool(name="outst", bufs=2))
            with tc.tile_pool(name="ps_o", bufs=2, space="PSUM") as ps_o:
                for fc in range(HC):
                    for t in range(N_TC):
                        op = ps_o.tile([P, TC], F32, name="op")
                        for pair in range(N_PAIRS):
                            nc.tensor.matmul(
                                op[:],
                                wo_sb[:, pair, ts(fc, P)],
                                o_sb[pair][:, ts(t, TC)],
                                start=(pair == 0),
                                stop=(pair == N_PAIRS - 1),
                            )
                        ob = outst.tile([P, TC], F32, name="ob")
                        nc.vector.tensor_copy(ob[:], op[:])
                        nc.sync.dma_start(out_d[ts(fc, P), ts(t, TC)], ob[:])

    nc.compile()
    return nc, names


_CACHE = {}


def _get_program():
    if "prog" not in _CACHE:
        _CACHE["prog"] = build_program()
    return _CACHE["prog"]


def _rope_tables():
    inv_freq = 1.0 / (BASE ** (np.arange(0, HD, 2, dtype=np.float64) / HD))
    t = np.arange(L, dtype=np.float64)
    freqs = np.outer(t, inv_freq)            # [L, 32]
    emb = np.concatenate((freqs, freqs), -1)  # [L, 64]
    cos = np.cos(emb).T.astype(np.float32)    # [64, L]
    sin = np.sin(emb).T.astype(np.float32)    # [64, L]
    sin_signed = sin.copy()
    sin_signed[: HD // 2] *= -1.0             # rotate_half sign baked in
    cosT = np.ascontiguousarray(np.concatenate([cos, cos], 0))      # [128, L]
    sinT = np.ascontiguousarray(np.concatenate([sin_signed, sin_signed], 0))
    return cosT, sinT


def make_in_maps(names, x, Wq, Wk, Wv, Wo):
    cosT, sinT = _rope_tables()
    in_maps = []
    xTs = [np.ascontiguousarray(x[b].T) for b in range(B)]
    for core in range(8):
        b = core // 4
        g = core % 4
        es = slice(g * E_LOCAL, (g + 1) * E_LOCAL)
        m = {
            names["in"][0]: xTs[b],
            names["in"][1]: np.ascontiguousarray(Wq[es, :].T),   # [1024, 256]
            names["in"][2]: np.ascontiguousarray(Wk[es, :].T),
            names["in"][3]: np.ascontiguousarray(Wv[es, :].T),
            names["in"][4]: np.ascontiguousarray(Wo[:, es].T),   # [256, 1024]
            names["in"][5]: cosT,
            names["in"][6]: sinT,
        }
        in_maps.append(m)
    return in_maps


def gather_out(names, res):
    out = np.zeros((B, L, HIDDEN), dtype=np.float32)
    for b in range(B):
        acc = np.zeros((HIDDEN, L), dtype=np.float32)
        for g in range(4):
            acc += res.results[b * 4 + g][names["out"]]
        out[b] = acc.T
    return out


def kernel(x, Wq, Wk, Wv, Wo):
    x = np.asarray(x, dtype=np.float32)
    Wq = np.asarray(Wq, dtype=np.float32)
    Wk = np.asarray(Wk, dtype=np.float32)
    Wv = np.asarray(Wv, dtype=np.float32)
    Wo = np.asarray(Wo, dtype=np.float32)

    nc, names = _get_program()
    in_maps = make_in_maps(names, x, Wq, Wk, Wv, Wo)
    res = run_bass_kernel_spmd(nc, in_maps, core_ids=list(range(8)))
    return gather_out(names, res)
